# revision 37
# baseline (speedup 1.0000x reference)
"""Sparse-attention wrapper kernel for 8 trn2 NeuronCores.

Sharding: core c -> (b = c // 4, g = c % 4). Data-parallel over batch B=2,
tensor-parallel over the 4 KV head groups (4 q-heads / 1 kv-head each).

v3: full-bf16 matmul pipeline, phase-ordered to keep the PE warm. Hidden
states are pre-scaled on the host by the ln-RMS reciprocals (f64), w_ln is
folded into the weights, w_qn/w_kn + RoPE rotation folded into cos/sin factor
tensors. Per-core:
  A) kT/vT = Wk/Wv^T @ hT streamed over d-chunks (hqT prefetched alongside);
     k-norm + RoPE as column scales; v transposed into bf16 stationaries.
  Q) qT = Wq^T @ hqT from resident tiles, split m{0,1}/m{2,3} so PSUM banks
     recycle without stalling; q-norm chains stage-parallel across heads.
  B) per-head attention per K-half, transposed orientation: scoresT in PSUM,
     one exp per s-tile, boundary masks, attn@v + ones row-sums, late recip.
  C) o_proj in 256-col chunks; the K-half-0 chunks are interleaved into the
     B half-1 head loop to fill PE gaps; each chunk ReduceScatters (bf16)
     directly into the output param. Host scatters K rows into [B,S,D] + bo.
"""

import numpy as np
import ml_dtypes
import concourse.bacc as bacc
import concourse.tile as tile
from concourse import mybir
from concourse.bass_utils import run_bass_kernel_spmd

B, S, K, D, H, HKV, HD = 2, 2048, 1024, 2048, 16, 4, 128
EPS = 1e-6
SCALE = float(HD) ** -0.5
NCORES = 8
NT = S // 128          # 16 s-tiles
NDC = D // 128         # 16 d-chunks
QH = H // HKV          # 4 q-heads per core
GW = QH * HD           # 512 columns of Wq per core
NCH = 4                # o_proj / ReduceScatter K-chunks
CW = K // NCH          # 256 columns per chunk
D4 = D // 4

F32 = mybir.dt.float32
BF16 = mybir.dt.bfloat16
AFT = mybir.ActivationFunctionType
BF = ml_dtypes.bfloat16

_BUILD_CACHE = {}
_LAST_IN_MAPS = None


def _build(klo_u, khi_max):
    nc = bacc.Bacc("TRN2", target_bir_lowering=False, debug=False,
                   num_devices=NCORES)

    mw = [max(0, khi_max[t] - klo_u[t]) for t in range(NT)]
    moff = np.concatenate([[0], np.cumsum(mw)]).astype(int)
    MW = int(moff[-1])

    p = {}
    p["hT"] = nc.declare_dram_parameter("hT", [D, S], BF16, isOutput=False)
    p["hqT"] = nc.declare_dram_parameter("hqT", [D, K], BF16, isOutput=False)
    p["wq"] = nc.declare_dram_parameter("wq", [128, NDC * GW], BF16,
                                        isOutput=False)
    p["wk"] = nc.declare_dram_parameter("wk", [128, D], BF16, isOutput=False)
    p["wv"] = nc.declare_dram_parameter("wv", [128, D], BF16, isOutput=False)
    p["wo"] = nc.declare_dram_parameter("wo", [GW, D], BF16, isOutput=False)
    p["cosq"] = nc.declare_dram_parameter("cosq", [HD, K], BF16,
                                          isOutput=False)
    p["sinq"] = nc.declare_dram_parameter("sinq", [HD, K], BF16,
                                          isOutput=False)
    p["cosk"] = nc.declare_dram_parameter("cosk", [HD, S], BF16,
                                          isOutput=False)
    p["sink"] = nc.declare_dram_parameter("sink", [HD, S], BF16,
                                          isOutput=False)
    p["maskp"] = nc.declare_dram_parameter("maskp", [128, max(MW, 1)], BF16,
                                           isOutput=False)
    p["ones128h"] = nc.declare_dram_parameter("ones128h", [128, 128], BF16,
                                              isOutput=False)
    p["epsp"] = nc.declare_dram_parameter("epsp", [128, 1], F32,
                                          isOutput=False)
    p["oshard"] = nc.declare_dram_parameter("oshard", [D4, K], BF16,
                                            isOutput=True)

    with tile.TileContext(nc) as tc:
        _emit(nc, tc, p, klo_u, khi_max, moff)
    nc.finalize()
    return nc


def _emit(nc, tc, p, klo_u, khi_max, moff):
    pool = lambda name, bufs=1, space="SBUF": tc.tile_pool(
        name=name, bufs=bufs, space=space)
    mw = [max(0, khi_max[t] - klo_u[t]) for t in range(NT)]

    with (
        pool("const") as constp,
        pool("persist") as persist,
        pool("wop") as wop,
        pool("dram", space="DRAM") as dramp,
    ):
        onesh_sb = constp.tile([128, 128], BF16, name="onesh_sb")
        eps_sb = constp.tile([128, 1], F32, name="eps_sb")
        cosq_sb = constp.tile([HD, K], BF16, name="cosq_sb")
        sinq_sb = constp.tile([HD, K], BF16, name="sinq_sb")
        cosk_sb = constp.tile([HD, S], BF16, name="cosk_sb")
        sink_sb = constp.tile([HD, S], BF16, name="sink_sb")
        mask_sb = constp.tile([128, max(int(moff[-1]), 1)], BF16,
                              name="mask_sb")

        kT_sb = persist.tile([HD, S], BF16, name="kT_sb")
        v_sb = [persist.tile([128, HD], BF16, tag=f"v{t}", name=f"v{t}")
                for t in range(NT)]
        qT_sb = [persist.tile([HD, K], BF16, tag=f"q{m}", name=f"q{m}")
                 for m in range(QH)]
        outT_sb = [persist.tile([HD, K], BF16, tag=f"o{m}", name=f"om{m}")
                   for m in range(QH)]
        hq_sb = [persist.tile([128, K], BF16, tag=f"hq{dc}", name=f"hq{dc}")
                 for dc in range(NDC)]
        wo_sb = [wop.tile([128, D], BF16, tag=f"wo{m}", name=f"wo{m}")
                 for m in range(QH)]
        wq_sb = wop.tile([128, NDC * GW], BF16, name="wq_sb")

        # ---------------- Phase A: k/v projections + k-norm + rope -------
        with (
            pool("wkv") as wkvp,
            pool("ha", bufs=4) as hap,
            pool("sqa", bufs=1) as sqp,
            pool("rowa", bufs=1) as rowp,
            pool("pA", bufs=1, space="PSUM") as pA,
        ):
            # queue split: sync = hT stream; vector = hqT stream; scalar =
            # weights in first-use order (wk/wv now, wq ~35us, wo ~150us)
            wk_sb = wkvp.tile([128, D], BF16, name="wk_sb")
            wv_sb = wkvp.tile([128, D], BF16, name="wv_sb")
            nc.scalar.dma_start(wk_sb[:], p["wk"][:])
            nc.scalar.dma_start(wv_sb[:], p["wv"][:])
            kraw = pA.tile([128, S], F32, tag="kraw", name="kraw")
            # v is projected straight into [s, hd] layout: per s-tile PSUM
            # regions (4 per bank), stationary = ht slice, moving = Wv chunk.
            # No transpose pass, and the banks free right after 16 small
            # copies -> phase Q's first head pair starts almost immediately.
            vp = [pA.tile([128, 512], F32, tag=f"vp{j}", name=f"vp{j}")
                  for j in range(4)]
            # 2-d-chunk super-tiles halve per-DMA overhead so the ht stream
            # outruns the PE (keeps the HAM clock-gate warm); consts, wq and
            # hqT follow on the same queue so they never compete with ht.
            for dc2 in range(NDC // 2):
                ht = hap.tile([128, 2 * S], BF16, tag="ht", name="ht")
                nc.sync.dma_start(
                    ht[:].rearrange("p (b s) -> p b s", b=2),
                    p["hT"][dc2 * 256:(dc2 + 1) * 256, :]
                    .rearrange("(b p) s -> p b s", b=2))
                for sub in range(2):
                    dc = 2 * dc2 + sub
                    for (a, b) in ((0, 512), (512, 1024), (1024, 1536),
                                   (1536, 2048)):
                        nc.tensor.matmul(kraw[:, a:b],
                                         wk_sb[:, dc * HD:(dc + 1) * HD],
                                         ht[:, sub * S + a:sub * S + b],
                                         start=(dc == 0),
                                         stop=(dc == NDC - 1))
                    for st in range(NT):
                        # start only on the first region per bank: start
                        # marks the whole 2KB bank pending-zero, so later
                        # regions' first writes zero-fill without it
                        nc.tensor.matmul(
                            vp[st // 4][:, (st % 4) * 128:(st % 4 + 1) * 128],
                            ht[:, sub * S + st * 128:sub * S + (st + 1) * 128],
                            wv_sb[:, dc * HD:(dc + 1) * HD],
                            start=(dc == 0 and st % 4 == 0),
                            stop=(dc == NDC - 1), skip_group_check=True)
            nc.sync.dma_start(onesh_sb[:], p["ones128h"][:])
            nc.sync.dma_start(eps_sb[:], p["epsp"][:])
            nc.sync.dma_start(cosk_sb[:], p["cosk"][:])
            nc.sync.dma_start(sink_sb[:], p["sink"][:])
            nc.sync.dma_start(cosq_sb[:], p["cosq"][:])
            nc.sync.dma_start(sinq_sb[:], p["sinq"][:])
            nc.sync.dma_start(mask_sb[:], p["maskp"][:])
            nc.sync.dma_start(wq_sb[:], p["wq"][:])
            for dc in range(NDC):
                nc.sync.dma_start(hq_sb[dc][:],
                                  p["hqT"][dc * 128:(dc + 1) * 128, :])
            # drain v PSUM first: q m0/m1 reuse those banks after the copies
            for j in range(NT):
                nc.vector.tensor_copy(
                    v_sb[j][:],
                    vp[j // 4][:, (j % 4) * 128:(j % 4 + 1) * 128])
            sqk = sqp.tile([128, S], BF16, tag="sqk", name="sqk")
            nc.scalar.square(sqk[:], kraw[:])
            # rope reads kraw, then msqk/vps recycle kraw's banks (vraw's
            # stay free for phase Q's first head pair)
            kc_ = rowp.tile([128, S], F32, tag="kc", name="kc_")
            nc.vector.tensor_mul(kc_[:], kraw[:], cosk_sb[:])
            ks = rowp.tile([128, S], F32, tag="ks", name="ks")
            nc.vector.tensor_mul(ks[0:64, :], kraw[64:128, :],
                                 sink_sb[0:64, :])
            nc.vector.tensor_mul(ks[64:128, :], kraw[0:64, :],
                                 sink_sb[64:128, :])
            nc.vector.tensor_add(kc_[:], kc_[:], ks[:])
            msqk = pA.tile([128, S], F32, tag="kraw", name="msqk")
            for (a, b) in ((0, 512), (512, 1024), (1024, 1536), (1536, 2048)):
                nc.tensor.matmul(msqk[:, a:b], onesh_sb[:], sqk[:, a:b],
                                 start=True, stop=True)
            t3 = rowp.tile([128, S], F32, tag="t3k", name="t3k")
            nc.scalar.activation(t3[:], msqk[:], AFT.Sqrt,
                                 bias=eps_sb[:], scale=1.0 / HD)
            comb = rowp.tile([128, S], F32, tag="combk", name="combk")
            nc.vector.reciprocal_approx_fast(comb[:], t3[:])
            nc.vector.tensor_mul(kT_sb[:], kc_[:], comb[:])

        # ---------------- Phase Q: q projection + q-norm + rope ----------
        with (
            pool("sqb", bufs=1) as sqbp,
            pool("rowq", bufs=1) as rowqp,
            pool("pq", bufs=1, space="PSUM") as pq,
        ):
            # allocation order [2,3,0,1]: m2/m3 land on kraw's banks (freed
            # late, used late), m0/m1 on vraw's (freed early, used first).
            qraw = [None] * QH
            for m in (2, 3, 0, 1):
                qraw[m] = pq.tile([128, K], F32, tag=f"qraw{m}",
                                  name=f"qraw{m}")
            sqm = [None] * QH
            qc = [None] * QH
            msqq = [None] * QH
            t3q = [None] * QH
            combq = [None] * QH

            def qchain_front(m):
                # square + rope + msq for head m (PE part lands in FIFO now)
                sqm[m] = sqbp.tile([128, K], BF16, tag=f"sqm{m}",
                                   name=f"sqm{m}")
                nc.scalar.square(sqm[m][:], qraw[m][:])
                qc[m] = rowqp.tile([128, K], F32, tag=f"qc{m}",
                                   name=f"qc{m}")
                nc.vector.tensor_mul(qc[m][:], qraw[m][:], cosq_sb[:])
                qs = rowqp.tile([128, K], F32, tag="qs", name="qs")
                nc.vector.tensor_mul(qs[0:64, :], qraw[m][64:128, :],
                                     sinq_sb[0:64, :])
                nc.vector.tensor_mul(qs[64:128, :], qraw[m][0:64, :],
                                     sinq_sb[64:128, :])
                nc.vector.tensor_add(qc[m][:], qc[m][:], qs[:])
                msqq[m] = pq.tile([128, K], F32, tag=f"qraw{m}",
                                  name=f"msqq{m}")
                for (a, b) in ((0, 512), (512, 1024)):
                    nc.tensor.matmul(msqq[m][:, a:b], onesh_sb[:],
                                     sqm[m][:, a:b], start=True, stop=True)

            for mpair in ((0, 1), (2, 3)):
                for dc in range(NDC):
                    for m in mpair:
                        for (a, b) in ((0, 512), (512, 1024)):
                            nc.tensor.matmul(
                                qraw[m][:, a:b],
                                wq_sb[:, dc * GW + m * HD:
                                      dc * GW + (m + 1) * HD],
                                hq_sb[dc][:, a:b], start=(dc == 0),
                                stop=(dc == NDC - 1))
                for m in mpair:
                    qchain_front(m)
                for m in mpair:
                    t3q[m] = rowqp.tile([128, K], F32, tag=f"t3q{m}",
                                        name=f"t3q{m}")
                    nc.scalar.activation(t3q[m][:], msqq[m][:], AFT.Sqrt,
                                         bias=eps_sb[:], scale=1.0 / HD)
                    combq[m] = rowqp.tile([128, K], F32, tag=f"combq{m}",
                                          name=f"combq{m}")
                    nc.vector.reciprocal_approx_fast(combq[m][:], t3q[m][:])
                    nc.vector.tensor_mul(qT_sb[m][:], qc[m][:], combq[m][:])
            # preload the exp table set while the PE winds down phase Q
            dummy = sqbp.tile([128, 1], F32, tag="dummy", name="dummy")
            nc.scalar.activation(dummy[:], eps_sb[:], AFT.Exp, scale=1.0)

        # ------- Phase B + C: attention per K-half, chunked o_proj/RS ----
        with (
            pool("expp", bufs=2) as expp,
            pool("rowb", bufs=2) as rowbp,
            pool("oevict", bufs=8) as oev,
            # open order fixes PSUM placement: pro/poT take qraw2/3's banks
            # (freed late, first written late); psc lands on qraw0/1's,
            # which free right after the first head pair's chains.
            pool("pro", bufs=1, space="PSUM") as pro,
            pool("poT", bufs=2, space="PSUM") as poT,
            pool("psc", bufs=2, space="PSUM") as psc,
        ):
            # wo arrives into the post-phase-A bandwidth hole
            for m in range(QH):
                nc.scalar.dma_start(wo_sb[m][:],
                                    p["wo"][m * 128:(m + 1) * 128, :])
            # chunks: two 256-col chunks overlap B half-1; two tail chunks
            # pipeline o_proj against the collectives
            CHB = [(0, 256), (256, 512), (512, 768), (768, 1024)]
            o_part = [dramp.tile([D, cb - ca], BF16, tag=f"opart{c}",
                                 name=f"opart{c}")
                      for c, (ca, cb) in enumerate(CHB)]
            o_shh = [dramp.tile([D4, cb - ca], BF16, tag=f"oshh{c}",
                                name=f"oshh{c}")
                     for c, (ca, cb) in enumerate(CHB)]

            def oproj_chunk(c, dclist):
                ca, cb = CHB[c]
                for dc in dclist:
                    ops = poT.tile([128, cb - ca], F32, tag="ops",
                                   name="ops")
                    for m2 in range(QH):
                        nc.tensor.matmul(
                            ops[:],
                            wo_sb[m2][:, dc * 128:(dc + 1) * 128],
                            outT_sb[m2][:, ca:cb],
                            start=(m2 == 0), stop=(m2 == QH - 1))
                    osb = oev.tile([128, cb - ca], BF16, tag="osb",
                                   name="osb")
                    nc.vector.tensor_copy(osb[:], ops[:])
                    nc.sync.dma_start(
                        o_part[c][dc * 128:(dc + 1) * 128, :], osb[:])

            def rs_chunk(c):
                nc.gpsimd.collective_compute(
                    "ReduceScatter", mybir.AluOpType.add,
                    replica_groups=[[0, 1, 2, 3], [4, 5, 6, 7]],
                    ins=[o_part[c].opt()], outs=[o_shh[c].opt()])

            def oshard_copy(c):
                ca, cb = CHB[c]
                nc.scalar.dma_start(p["oshard"][:, ca:cb], o_shh[c][:])

            for kh in range(2):
                klo_h, khi_h = kh * 512, (kh + 1) * 512
                act_t = [t for t in range(NT) if klo_u[t] < khi_h]
                first_t, last_t = act_t[0], act_t[-1]
                pairs = [tuple(act_t[i:i + 2])
                         for i in range(0, len(act_t), 2)]
                for m in range(QH):
                    ets = {}
                    for pr in pairs:
                        # pack the pair flush at the bank boundary: t0 in
                        # [lo0-klo_h, 512), t1 in [512, 512+w1) -> one exp
                        t0 = pr[0]
                        lo0 = max(klo_u[t0], klo_h)
                        w0 = khi_h - lo0
                        sc = psc.tile([128, 1024], F32, tag="sc", name="sc")
                        nc.tensor.matmul(
                            sc[:, lo0 - klo_h:512],
                            kT_sb[:, t0 * 128:(t0 + 1) * 128],
                            qT_sb[m][:, lo0:khi_h], start=True, stop=True)
                        wtot = w0
                        if len(pr) == 2:
                            t1 = pr[1]
                            lo1 = max(klo_u[t1], klo_h)
                            w1 = khi_h - lo1
                            nc.tensor.matmul(
                                sc[:, 512:512 + w1],
                                kT_sb[:, t1 * 128:(t1 + 1) * 128],
                                qT_sb[m][:, lo1:khi_h], start=True,
                                stop=True)
                            wtot = w0 + w1
                        et = expp.tile([128, wtot], BF16,
                                       tag=f"e{t0}", name=f"e{t0}")
                        nc.scalar.activation(
                            et[:], sc[:, lo0 - klo_h:lo0 - klo_h + wtot],
                            AFT.Exp, scale=SCALE)
                        ets[t0] = (et, 0)
                        if len(pr) == 2:
                            ets[pr[1]] = (et, w0)
                        for t in pr:
                            lo = max(klo_u[t], klo_h)
                            off = ets[t][1]
                            hi_m = min(khi_max[t], khi_h)
                            if hi_m > lo:
                                mo = int(moff[t]) + (lo - klo_u[t])
                                w = hi_m - lo
                                nc.vector.tensor_mul(
                                    et[:, off:off + w], et[:, off:off + w],
                                    mask_sb[:, mo:mo + w])
                    # fill the PE while ACT runs the exps: o_proj quarters
                    # of the previous K-half
                    if kh == 1:
                        qn = m
                        oproj_chunk(qn // 2,
                                    range(8 * (qn % 2), 8 * (qn % 2) + 8))
                        if qn == 1:
                            rs_chunk(0)
                        elif qn == 3:
                            rs_chunk(1)
                    rsum = pro.tile([128, 512], F32, tag="rsum", name="rsum")
                    for t in act_t:
                        lo = max(klo_u[t], klo_h)
                        et, off = ets[t]
                        nc.tensor.matmul(rsum[:, lo - klo_h:], onesh_sb[:],
                                         et[:, off:off + khi_h - lo],
                                         start=(t == first_t),
                                         stop=(t == last_t))
                    outp = pro.tile([HD, 512], F32, tag="outp", name="outp")
                    for t in act_t:
                        lo = max(klo_u[t], klo_h)
                        et, off = ets[t]
                        nc.tensor.matmul(outp[:, lo - klo_h:], v_sb[t][:],
                                         et[:, off:off + khi_h - lo],
                                         start=(t == first_t),
                                         stop=(t == last_t))
                    recip = rowbp.tile([128, 512], F32, tag="recip",
                                       name="recip")
                    nc.vector.reciprocal_approx_fast(recip[:], rsum[:])
                    nc.vector.tensor_mul(outT_sb[m][:, klo_h:khi_h],
                                         outp[:], recip[:])
            # tail: o_proj c3 overlaps the c2 collective; copies last
            oproj_chunk(2, range(NDC))
            rs_chunk(2)
            oproj_chunk(3, range(NDC))
            rs_chunk(3)
            for c in range(len(CHB)):
                oshard_copy(c)


def kernel(hidden_states, pos_ids, cos, sin, w_ln, w_qn, w_kn,
           Wq, Wk, Wv, Wo, bo):
    h = np.asarray(hidden_states, dtype=np.float64)
    pos = np.asarray(pos_ids)
    cos0 = np.asarray(cos, dtype=np.float64)[0]          # [S, HD]
    sin0 = np.asarray(sin, dtype=np.float64)[0]
    w_ln = np.asarray(w_ln, dtype=np.float64)
    w_qn = np.asarray(w_qn, dtype=np.float64)
    w_kn = np.asarray(w_kn, dtype=np.float64)
    Wq = np.asarray(Wq, dtype=np.float64)
    Wk = np.asarray(Wk, dtype=np.float64)
    Wv = np.asarray(Wv, dtype=np.float64)
    Wo = np.asarray(Wo, dtype=np.float32)
    bo = np.asarray(bo, dtype=np.float32)

    order = np.argsort(pos, axis=1, kind="stable")
    pos_s = np.take_along_axis(pos, order, axis=1)       # sorted per batch

    klo = np.stack([np.searchsorted(pos_s[b], np.arange(NT + 1) * 128)
                    for b in range(B)])                   # [B, NT+1]
    # PSUM matmul dst offsets must stay 8-element aligned.
    klo_u = ((klo[:, :NT].min(axis=0) // 8) * 8).astype(int).tolist()
    khi_max = klo[:, 1:].max(axis=0).astype(int).tolist()

    key = (tuple(klo_u), tuple(khi_max))
    if key not in _BUILD_CACHE:
        _BUILD_CACHE[key] = _build(klo_u, khi_max)
    nc = _BUILD_CACHE[key]

    Wq_f = w_ln[:, None] * Wq
    Wk_f = w_ln[:, None] * Wk
    Wv_f = w_ln[:, None] * Wv

    sgn = np.where(np.arange(HD) < 64, -1.0, 1.0)[:, None]
    wqn_sh = np.roll(w_qn, -64)[:, None]
    wkn_sh = np.roll(w_kn, -64)[:, None]
    COSK = np.ascontiguousarray((w_kn[:, None] * cos0.T).astype(BF))
    SINK = np.ascontiguousarray((wkn_sh * sin0.T * sgn).astype(BF))

    mw = [max(0, khi_max[t] - klo_u[t]) for t in range(NT)]
    moff = np.concatenate([[0], np.cumsum(mw)]).astype(int)
    MW = max(int(moff[-1]), 1)

    p_arange = np.arange(128)[:, None]
    rs_all = 1.0 / np.sqrt((h ** 2).mean(axis=2) + EPS)   # [B, S] f64
    hn = h * rs_all[:, :, None]                           # pre-normed, f64
    in_maps = []
    for c in range(NCORES):
        b, g = c // 4, c % 4
        ps = pos_s[b]
        hTb = np.ascontiguousarray(hn[b].T.astype(BF))
        hqTb = np.ascontiguousarray(hn[b][ps].T.astype(BF))
        COSQ = np.ascontiguousarray((w_qn[:, None] * cos0[ps].T).astype(BF))
        SINQ = np.ascontiguousarray((wqn_sh * sin0[ps].T * sgn).astype(BF))
        maskp = np.zeros((128, MW), dtype=BF)
        for t in range(NT):
            if mw[t] == 0:
                continue
            cols = ps[klo_u[t]:klo_u[t] + mw[t]][None, :]
            maskp[:, int(moff[t]):int(moff[t]) + mw[t]] = (
                (t * 128 + p_arange) <= cols).astype(BF)
        in_maps.append({
            "hT": hTb,
            "hqT": hqTb,
            "wq": np.ascontiguousarray(
                Wq_f[:, g * GW:(g + 1) * GW].reshape(NDC, 128, GW)
                .transpose(1, 0, 2).reshape(128, NDC * GW).astype(BF)),
            "wk": np.ascontiguousarray(
                Wk_f[:, g * HD:(g + 1) * HD].reshape(NDC, 128, HD)
                .transpose(1, 0, 2).reshape(128, D).astype(BF)),
            "wv": np.ascontiguousarray(
                Wv_f[:, g * HD:(g + 1) * HD].reshape(NDC, 128, HD)
                .transpose(1, 0, 2).reshape(128, D).astype(BF)),
            "wo": np.ascontiguousarray(Wo[g * GW:(g + 1) * GW, :].astype(BF)),
            "cosq": COSQ, "sinq": SINQ, "cosk": COSK, "sink": SINK,
            "maskp": maskp,
            "ones128h": np.ones((128, 128), dtype=BF),
            "epsp": np.full((128, 1), EPS, dtype=np.float32),
        })

    global _LAST_IN_MAPS
    _LAST_IN_MAPS = in_maps
    res = run_bass_kernel_spmd(nc, in_maps, list(range(NCORES)))

    out = np.zeros((B, S, D), dtype=np.float32)
    for b in range(B):
        oT = np.concatenate(
            [np.asarray(res.results[4 * b + g]["oshard"]).astype(np.float32)
             for g in range(4)], axis=0)
        out[b, pos_s[b], :] = oT.T + bo[None, :]
    return out


# revision 38
# speedup vs baseline: 1.0865x; 1.0865x over previous
"""Sparse-attention wrapper kernel for 8 trn2 NeuronCores.

Sharding: core c -> (b = c // 4, g = c % 4). Data-parallel over batch B=2,
tensor-parallel over the 4 KV head groups (4 q-heads / 1 kv-head each).

Full-bf16 matmul pipeline, phase-ordered to keep the PE clock-gate warm.
Hidden states are pre-scaled on the host by the ln-RMS reciprocals (f64),
w_ln is folded into the weights, w_qn/w_kn + RoPE rotation folded into
cos/sin factor tensors, bo added on the host. Per-core:
  A) kT = Wk^T @ hT streamed in 2-d-chunk super-tiles (DMA outruns the PE);
     v projected straight into [s, hd] PSUM regions (no transpose pass);
     k-norm + RoPE fused as column scales. Consts/wq/hqT queue behind the
     ht stream on sync so they never steal its bandwidth.
  Q) qT = Wq^T @ hqT from resident tiles, head-pairs split so PSUM banks
     recycle without stalls; q-norm chains pipelined per pair.
  B) per-head attention per K-half, transposed orientation: score s-tile
     PAIRS packed flush at the PSUM bank boundary so one exp covers both
     (halves ACT instruction overhead), boundary masks on DVE, attn@v +
     ones row-sums, late 1/rowsum.
  C) o_proj in K-chunks: chunks 0/1 interleaved into the B half-1 head loop
     (fills PE gaps), chunks 2/3 pipelined against their ReduceScatters
     (bf16); output copies drain last. Host scatters K rows into [B,S,D].
"""

import numpy as np
import ml_dtypes
import concourse.bacc as bacc
import concourse.tile as tile
from concourse import mybir
from concourse.bass_utils import run_bass_kernel_spmd

B, S, K, D, H, HKV, HD = 2, 2048, 1024, 2048, 16, 4, 128
EPS = 1e-6
SCALE = float(HD) ** -0.5
NCORES = 8
NT = S // 128          # 16 s-tiles
NDC = D // 128         # 16 d-chunks
QH = H // HKV          # 4 q-heads per core
GW = QH * HD           # 512 columns of Wq per core
NCH = 4                # o_proj / ReduceScatter K-chunks
CW = K // NCH          # 256 columns per chunk
D4 = D // 4

F32 = mybir.dt.float32
BF16 = mybir.dt.bfloat16
AFT = mybir.ActivationFunctionType
BF = ml_dtypes.bfloat16

_BUILD_CACHE = {}
_LAST_IN_MAPS = None


def _build(klo_u, khi_max):
    nc = bacc.Bacc("TRN2", target_bir_lowering=False, debug=False,
                   num_devices=NCORES)

    mw = [max(0, khi_max[t] - klo_u[t]) for t in range(NT)]
    moff = np.concatenate([[0], np.cumsum(mw)]).astype(int)
    MW = int(moff[-1])

    p = {}
    p["hT"] = nc.declare_dram_parameter("hT", [D, S], BF16, isOutput=False)
    p["hqT"] = nc.declare_dram_parameter("hqT", [D, K], BF16, isOutput=False)
    p["wq"] = nc.declare_dram_parameter("wq", [128, NDC * GW], BF16,
                                        isOutput=False)
    p["wk"] = nc.declare_dram_parameter("wk", [128, D], BF16, isOutput=False)
    p["wv"] = nc.declare_dram_parameter("wv", [128, D], BF16, isOutput=False)
    p["wo"] = nc.declare_dram_parameter("wo", [GW, D], BF16, isOutput=False)
    p["cosq"] = nc.declare_dram_parameter("cosq", [HD, K], BF16,
                                          isOutput=False)
    p["sinq"] = nc.declare_dram_parameter("sinq", [HD, K], BF16,
                                          isOutput=False)
    p["cosk"] = nc.declare_dram_parameter("cosk", [HD, S], BF16,
                                          isOutput=False)
    p["sink"] = nc.declare_dram_parameter("sink", [HD, S], BF16,
                                          isOutput=False)
    p["maskp"] = nc.declare_dram_parameter("maskp", [128, max(MW, 1)], BF16,
                                           isOutput=False)
    p["ones128h"] = nc.declare_dram_parameter("ones128h", [128, 128], BF16,
                                              isOutput=False)
    p["epsp"] = nc.declare_dram_parameter("epsp", [128, 1], F32,
                                          isOutput=False)
    p["oshard"] = nc.declare_dram_parameter("oshard", [D4, K], BF16,
                                            isOutput=True)

    with tile.TileContext(nc) as tc:
        _emit(nc, tc, p, klo_u, khi_max, moff)
    nc.finalize()
    return nc


def _emit(nc, tc, p, klo_u, khi_max, moff):
    pool = lambda name, bufs=1, space="SBUF": tc.tile_pool(
        name=name, bufs=bufs, space=space)
    mw = [max(0, khi_max[t] - klo_u[t]) for t in range(NT)]

    with (
        pool("const") as constp,
        pool("persist") as persist,
        pool("wop") as wop,
        pool("dram", space="DRAM") as dramp,
    ):
        onesh_sb = constp.tile([128, 128], BF16, name="onesh_sb")
        eps_sb = constp.tile([128, 1], F32, name="eps_sb")
        cosq_sb = constp.tile([HD, K], BF16, name="cosq_sb")
        sinq_sb = constp.tile([HD, K], BF16, name="sinq_sb")
        cosk_sb = constp.tile([HD, S], BF16, name="cosk_sb")
        sink_sb = constp.tile([HD, S], BF16, name="sink_sb")
        mask_sb = constp.tile([128, max(int(moff[-1]), 1)], BF16,
                              name="mask_sb")

        kT_sb = persist.tile([HD, S], BF16, name="kT_sb")
        v_sb = [persist.tile([128, HD], BF16, tag=f"v{t}", name=f"v{t}")
                for t in range(NT)]
        qT_sb = [persist.tile([HD, K], BF16, tag=f"q{m}", name=f"q{m}")
                 for m in range(QH)]
        outT_sb = [persist.tile([HD, K], BF16, tag=f"o{m}", name=f"om{m}")
                   for m in range(QH)]
        hq_sb = [persist.tile([128, K], BF16, tag=f"hq{dc}", name=f"hq{dc}")
                 for dc in range(NDC)]
        wo_sb = [wop.tile([128, D], BF16, tag=f"wo{m}", name=f"wo{m}")
                 for m in range(QH)]
        wq_sb = wop.tile([128, NDC * GW], BF16, name="wq_sb")

        # ---------------- Phase A: k/v projections + k-norm + rope -------
        with (
            pool("wkv") as wkvp,
            pool("ha", bufs=4) as hap,
            pool("sqa", bufs=1) as sqp,
            pool("rowa", bufs=1) as rowp,
            pool("pA", bufs=1, space="PSUM") as pA,
        ):
            # queue split: sync = hT stream; vector = hqT stream; scalar =
            # weights in first-use order (wk/wv now, wq ~35us, wo ~150us)
            wk_sb = wkvp.tile([128, D], BF16, name="wk_sb")
            wv_sb = wkvp.tile([128, D], BF16, name="wv_sb")
            nc.scalar.dma_start(wk_sb[:], p["wk"][:])
            nc.scalar.dma_start(wv_sb[:], p["wv"][:])
            kraw = pA.tile([128, S], F32, tag="kraw", name="kraw")
            # v is projected straight into [s, hd] layout: per s-tile PSUM
            # regions (4 per bank), stationary = ht slice, moving = Wv chunk.
            # No transpose pass, and the banks free right after 16 small
            # copies -> phase Q's first head pair starts almost immediately.
            vp = [pA.tile([128, 512], F32, tag=f"vp{j}", name=f"vp{j}")
                  for j in range(4)]
            # 2-d-chunk super-tiles halve per-DMA overhead so the ht stream
            # outruns the PE (keeps the HAM clock-gate warm); consts, wq and
            # hqT follow on the same queue so they never compete with ht.
            for dc2 in range(NDC // 2):
                ht = hap.tile([128, 2 * S], BF16, tag="ht", name="ht")
                nc.sync.dma_start(
                    ht[:].rearrange("p (b s) -> p b s", b=2),
                    p["hT"][dc2 * 256:(dc2 + 1) * 256, :]
                    .rearrange("(b p) s -> p b s", b=2))
                for sub in range(2):
                    dc = 2 * dc2 + sub
                    for (a, b) in ((0, 512), (512, 1024), (1024, 1536),
                                   (1536, 2048)):
                        nc.tensor.matmul(kraw[:, a:b],
                                         wk_sb[:, dc * HD:(dc + 1) * HD],
                                         ht[:, sub * S + a:sub * S + b],
                                         start=(dc == 0),
                                         stop=(dc == NDC - 1))
                    for st in range(NT):
                        # start only on the first region per bank: start
                        # marks the whole 2KB bank pending-zero, so later
                        # regions' first writes zero-fill without it
                        nc.tensor.matmul(
                            vp[st // 4][:, (st % 4) * 128:(st % 4 + 1) * 128],
                            ht[:, sub * S + st * 128:sub * S + (st + 1) * 128],
                            wv_sb[:, dc * HD:(dc + 1) * HD],
                            start=(dc == 0 and st % 4 == 0),
                            stop=(dc == NDC - 1), skip_group_check=True)
            nc.sync.dma_start(onesh_sb[:], p["ones128h"][:])
            nc.sync.dma_start(eps_sb[:], p["epsp"][:])
            nc.sync.dma_start(cosk_sb[:], p["cosk"][:])
            nc.sync.dma_start(sink_sb[:], p["sink"][:])
            nc.sync.dma_start(cosq_sb[:], p["cosq"][:])
            nc.sync.dma_start(sinq_sb[:], p["sinq"][:])
            nc.sync.dma_start(mask_sb[:], p["maskp"][:])
            nc.sync.dma_start(wq_sb[:], p["wq"][:])
            for dc in range(NDC):
                nc.sync.dma_start(hq_sb[dc][:],
                                  p["hqT"][dc * 128:(dc + 1) * 128, :])
            # drain v PSUM first: q m0/m1 reuse those banks after the copies
            for j in range(NT):
                nc.vector.tensor_copy(
                    v_sb[j][:],
                    vp[j // 4][:, (j % 4) * 128:(j % 4 + 1) * 128])
            sqk = sqp.tile([128, S], BF16, tag="sqk", name="sqk")
            nc.scalar.square(sqk[:], kraw[:])
            # rope reads kraw, then msqk/vps recycle kraw's banks (vraw's
            # stay free for phase Q's first head pair)
            kc_ = rowp.tile([128, S], F32, tag="kc", name="kc_")
            nc.vector.tensor_mul(kc_[:], kraw[:], cosk_sb[:])
            ks = rowp.tile([128, S], F32, tag="ks", name="ks")
            nc.vector.tensor_mul(ks[0:64, :], kraw[64:128, :],
                                 sink_sb[0:64, :])
            nc.vector.tensor_mul(ks[64:128, :], kraw[0:64, :],
                                 sink_sb[64:128, :])
            nc.vector.tensor_add(kc_[:], kc_[:], ks[:])
            msqk = pA.tile([128, S], F32, tag="kraw", name="msqk")
            for (a, b) in ((0, 512), (512, 1024), (1024, 1536), (1536, 2048)):
                nc.tensor.matmul(msqk[:, a:b], onesh_sb[:], sqk[:, a:b],
                                 start=True, stop=True)
            t3 = rowp.tile([128, S], F32, tag="t3k", name="t3k")
            nc.scalar.activation(t3[:], msqk[:], AFT.Sqrt,
                                 bias=eps_sb[:], scale=1.0 / HD)
            comb = rowp.tile([128, S], F32, tag="combk", name="combk")
            nc.vector.reciprocal_approx_fast(comb[:], t3[:])
            nc.vector.tensor_mul(kT_sb[:], kc_[:], comb[:])

        # ---------------- Phase Q: q projection + q-norm + rope ----------
        with (
            pool("sqb", bufs=1) as sqbp,
            pool("rowq", bufs=1) as rowqp,
            pool("pq", bufs=1, space="PSUM") as pq,
        ):
            # allocation order [2,3,0,1]: m2/m3 land on kraw's banks (freed
            # late, used late), m0/m1 on vraw's (freed early, used first).
            qraw = [None] * QH
            for m in (2, 3, 0, 1):
                qraw[m] = pq.tile([128, K], F32, tag=f"qraw{m}",
                                  name=f"qraw{m}")
            sqm = [None] * QH
            qc = [None] * QH
            msqq = [None] * QH
            t3q = [None] * QH
            combq = [None] * QH

            def qchain_front(m):
                # square + rope + msq for head m (PE part lands in FIFO now)
                sqm[m] = sqbp.tile([128, K], BF16, tag=f"sqm{m}",
                                   name=f"sqm{m}")
                nc.scalar.square(sqm[m][:], qraw[m][:])
                qc[m] = rowqp.tile([128, K], F32, tag=f"qc{m}",
                                   name=f"qc{m}")
                nc.vector.tensor_mul(qc[m][:], qraw[m][:], cosq_sb[:])
                qs = rowqp.tile([128, K], F32, tag="qs", name="qs")
                nc.vector.tensor_mul(qs[0:64, :], qraw[m][64:128, :],
                                     sinq_sb[0:64, :])
                nc.vector.tensor_mul(qs[64:128, :], qraw[m][0:64, :],
                                     sinq_sb[64:128, :])
                nc.vector.tensor_add(qc[m][:], qc[m][:], qs[:])
                msqq[m] = pq.tile([128, K], F32, tag=f"qraw{m}",
                                  name=f"msqq{m}")
                for (a, b) in ((0, 512), (512, 1024)):
                    nc.tensor.matmul(msqq[m][:, a:b], onesh_sb[:],
                                     sqm[m][:, a:b], start=True, stop=True)

            for mpair in ((0, 1), (2, 3)):
                for dc in range(NDC):
                    for m in mpair:
                        for (a, b) in ((0, 512), (512, 1024)):
                            nc.tensor.matmul(
                                qraw[m][:, a:b],
                                wq_sb[:, dc * GW + m * HD:
                                      dc * GW + (m + 1) * HD],
                                hq_sb[dc][:, a:b], start=(dc == 0),
                                stop=(dc == NDC - 1))
                for m in mpair:
                    qchain_front(m)
                for m in mpair:
                    t3q[m] = rowqp.tile([128, K], F32, tag=f"t3q{m}",
                                        name=f"t3q{m}")
                    nc.scalar.activation(t3q[m][:], msqq[m][:], AFT.Sqrt,
                                         bias=eps_sb[:], scale=1.0 / HD)
                    combq[m] = rowqp.tile([128, K], F32, tag=f"combq{m}",
                                          name=f"combq{m}")
                    nc.vector.reciprocal_approx_fast(combq[m][:], t3q[m][:])
                    nc.vector.tensor_mul(qT_sb[m][:], qc[m][:], combq[m][:])
            # preload the exp table set while the PE winds down phase Q
            dummy = sqbp.tile([128, 1], F32, tag="dummy", name="dummy")
            nc.scalar.activation(dummy[:], eps_sb[:], AFT.Exp, scale=1.0)

        # ------- Phase B + C: attention per K-half, chunked o_proj/RS ----
        with (
            pool("expp", bufs=2) as expp,
            pool("rowb", bufs=2) as rowbp,
            pool("oevict", bufs=8) as oev,
            # open order fixes PSUM placement: pro/poT take qraw2/3's banks
            # (freed late, first written late); psc lands on qraw0/1's,
            # which free right after the first head pair's chains.
            pool("pro", bufs=1, space="PSUM") as pro,
            pool("poT", bufs=2, space="PSUM") as poT,
            pool("psc", bufs=2, space="PSUM") as psc,
        ):
            # wo arrives into the post-phase-A bandwidth hole
            for m in range(QH):
                nc.scalar.dma_start(wo_sb[m][:],
                                    p["wo"][m * 128:(m + 1) * 128, :])
            # chunks: two 256-col chunks overlap B half-1; two tail chunks
            # pipeline o_proj against the collectives
            CHB = [(0, 256), (256, 512), (512, 768), (768, 1024)]
            o_part = [dramp.tile([D, cb - ca], BF16, tag=f"opart{c}",
                                 name=f"opart{c}")
                      for c, (ca, cb) in enumerate(CHB)]
            o_shh = [dramp.tile([D4, cb - ca], BF16, tag=f"oshh{c}",
                                name=f"oshh{c}")
                     for c, (ca, cb) in enumerate(CHB)]

            def oproj_chunk(c, dclist):
                ca, cb = CHB[c]
                for dc in dclist:
                    ops = poT.tile([128, cb - ca], F32, tag="ops",
                                   name="ops")
                    for m2 in range(QH):
                        nc.tensor.matmul(
                            ops[:],
                            wo_sb[m2][:, dc * 128:(dc + 1) * 128],
                            outT_sb[m2][:, ca:cb],
                            start=(m2 == 0), stop=(m2 == QH - 1))
                    osb = oev.tile([128, cb - ca], BF16, tag="osb",
                                   name="osb")
                    nc.vector.tensor_copy(osb[:], ops[:])
                    nc.sync.dma_start(
                        o_part[c][dc * 128:(dc + 1) * 128, :], osb[:])

            def rs_chunk(c):
                nc.gpsimd.collective_compute(
                    "ReduceScatter", mybir.AluOpType.add,
                    replica_groups=[[0, 1, 2, 3], [4, 5, 6, 7]],
                    ins=[o_part[c].opt()], outs=[o_shh[c].opt()])

            def oshard_copy(c):
                ca, cb = CHB[c]
                nc.scalar.dma_start(p["oshard"][:, ca:cb], o_shh[c][:])

            for kh in range(2):
                klo_h, khi_h = kh * 512, (kh + 1) * 512
                act_t = [t for t in range(NT) if klo_u[t] < khi_h]
                first_t, last_t = act_t[0], act_t[-1]
                pairs = [tuple(act_t[i:i + 2])
                         for i in range(0, len(act_t), 2)]
                for m in range(QH):
                    ets = {}
                    for pr in pairs:
                        # pack the pair flush at the bank boundary: t0 in
                        # [lo0-klo_h, 512), t1 in [512, 512+w1) -> one exp
                        t0 = pr[0]
                        lo0 = max(klo_u[t0], klo_h)
                        w0 = khi_h - lo0
                        sc = psc.tile([128, 1024], F32, tag="sc", name="sc")
                        nc.tensor.matmul(
                            sc[:, lo0 - klo_h:512],
                            kT_sb[:, t0 * 128:(t0 + 1) * 128],
                            qT_sb[m][:, lo0:khi_h], start=True, stop=True)
                        wtot = w0
                        if len(pr) == 2:
                            t1 = pr[1]
                            lo1 = max(klo_u[t1], klo_h)
                            w1 = khi_h - lo1
                            nc.tensor.matmul(
                                sc[:, 512:512 + w1],
                                kT_sb[:, t1 * 128:(t1 + 1) * 128],
                                qT_sb[m][:, lo1:khi_h], start=True,
                                stop=True)
                            wtot = w0 + w1
                        et = expp.tile([128, wtot], BF16,
                                       tag=f"e{t0}", name=f"e{t0}")
                        nc.scalar.activation(
                            et[:], sc[:, lo0 - klo_h:lo0 - klo_h + wtot],
                            AFT.Exp, scale=SCALE)
                        ets[t0] = (et, 0)
                        if len(pr) == 2:
                            ets[pr[1]] = (et, w0)
                        for t in pr:
                            lo = max(klo_u[t], klo_h)
                            off = ets[t][1]
                            hi_m = min(khi_max[t], khi_h)
                            if hi_m > lo:
                                mo = int(moff[t]) + (lo - klo_u[t])
                                w = hi_m - lo
                                nc.vector.tensor_mul(
                                    et[:, off:off + w], et[:, off:off + w],
                                    mask_sb[:, mo:mo + w])
                    # fill the PE while ACT runs the exps: o_proj quarters
                    # of the previous K-half
                    if kh == 1:
                        qn = m
                        oproj_chunk(qn // 2,
                                    range(8 * (qn % 2), 8 * (qn % 2) + 8))
                        if qn == 1:
                            rs_chunk(0)
                        elif qn == 3:
                            rs_chunk(1)
                    rsum = pro.tile([128, 512], F32, tag="rsum", name="rsum")
                    for t in act_t:
                        lo = max(klo_u[t], klo_h)
                        et, off = ets[t]
                        nc.tensor.matmul(rsum[:, lo - klo_h:], onesh_sb[:],
                                         et[:, off:off + khi_h - lo],
                                         start=(t == first_t),
                                         stop=(t == last_t))
                    outp = pro.tile([HD, 512], F32, tag="outp", name="outp")
                    for t in act_t:
                        lo = max(klo_u[t], klo_h)
                        et, off = ets[t]
                        nc.tensor.matmul(outp[:, lo - klo_h:], v_sb[t][:],
                                         et[:, off:off + khi_h - lo],
                                         start=(t == first_t),
                                         stop=(t == last_t))
                    recip = rowbp.tile([128, 512], F32, tag="recip",
                                       name="recip")
                    nc.vector.reciprocal_approx_fast(recip[:], rsum[:])
                    nc.vector.tensor_mul(outT_sb[m][:, klo_h:khi_h],
                                         outp[:], recip[:])
            # tail: o_proj c3 overlaps the c2 collective; copies last
            oproj_chunk(2, range(NDC))
            rs_chunk(2)
            oproj_chunk(3, range(NDC))
            rs_chunk(3)
            for c in range(len(CHB)):
                oshard_copy(c)


def kernel(hidden_states, pos_ids, cos, sin, w_ln, w_qn, w_kn,
           Wq, Wk, Wv, Wo, bo):
    h = np.asarray(hidden_states, dtype=np.float64)
    pos = np.asarray(pos_ids)
    cos0 = np.asarray(cos, dtype=np.float64)[0]          # [S, HD]
    sin0 = np.asarray(sin, dtype=np.float64)[0]
    w_ln = np.asarray(w_ln, dtype=np.float64)
    w_qn = np.asarray(w_qn, dtype=np.float64)
    w_kn = np.asarray(w_kn, dtype=np.float64)
    Wq = np.asarray(Wq, dtype=np.float64)
    Wk = np.asarray(Wk, dtype=np.float64)
    Wv = np.asarray(Wv, dtype=np.float64)
    Wo = np.asarray(Wo, dtype=np.float32)
    bo = np.asarray(bo, dtype=np.float32)

    order = np.argsort(pos, axis=1, kind="stable")
    pos_s = np.take_along_axis(pos, order, axis=1)       # sorted per batch

    klo = np.stack([np.searchsorted(pos_s[b], np.arange(NT + 1) * 128)
                    for b in range(B)])                   # [B, NT+1]
    # PSUM matmul dst offsets must stay 8-element aligned.
    klo_u = ((klo[:, :NT].min(axis=0) // 8) * 8).astype(int).tolist()
    khi_max = klo[:, 1:].max(axis=0).astype(int).tolist()

    key = (tuple(klo_u), tuple(khi_max))
    if key not in _BUILD_CACHE:
        _BUILD_CACHE[key] = _build(klo_u, khi_max)
    nc = _BUILD_CACHE[key]

    Wq_f = w_ln[:, None] * Wq
    Wk_f = w_ln[:, None] * Wk
    Wv_f = w_ln[:, None] * Wv

    sgn = np.where(np.arange(HD) < 64, -1.0, 1.0)[:, None]
    wqn_sh = np.roll(w_qn, -64)[:, None]
    wkn_sh = np.roll(w_kn, -64)[:, None]
    COSK = np.ascontiguousarray((w_kn[:, None] * cos0.T).astype(BF))
    SINK = np.ascontiguousarray((wkn_sh * sin0.T * sgn).astype(BF))

    mw = [max(0, khi_max[t] - klo_u[t]) for t in range(NT)]
    moff = np.concatenate([[0], np.cumsum(mw)]).astype(int)
    MW = max(int(moff[-1]), 1)

    p_arange = np.arange(128)[:, None]
    rs_all = 1.0 / np.sqrt((h ** 2).mean(axis=2) + EPS)   # [B, S] f64
    hn = h * rs_all[:, :, None]                           # pre-normed, f64
    in_maps = []
    for c in range(NCORES):
        b, g = c // 4, c % 4
        ps = pos_s[b]
        hTb = np.ascontiguousarray(hn[b].T.astype(BF))
        hqTb = np.ascontiguousarray(hn[b][ps].T.astype(BF))
        COSQ = np.ascontiguousarray((w_qn[:, None] * cos0[ps].T).astype(BF))
        SINQ = np.ascontiguousarray((wqn_sh * sin0[ps].T * sgn).astype(BF))
        maskp = np.zeros((128, MW), dtype=BF)
        for t in range(NT):
            if mw[t] == 0:
                continue
            cols = ps[klo_u[t]:klo_u[t] + mw[t]][None, :]
            maskp[:, int(moff[t]):int(moff[t]) + mw[t]] = (
                (t * 128 + p_arange) <= cols).astype(BF)
        in_maps.append({
            "hT": hTb,
            "hqT": hqTb,
            "wq": np.ascontiguousarray(
                Wq_f[:, g * GW:(g + 1) * GW].reshape(NDC, 128, GW)
                .transpose(1, 0, 2).reshape(128, NDC * GW).astype(BF)),
            "wk": np.ascontiguousarray(
                Wk_f[:, g * HD:(g + 1) * HD].reshape(NDC, 128, HD)
                .transpose(1, 0, 2).reshape(128, D).astype(BF)),
            "wv": np.ascontiguousarray(
                Wv_f[:, g * HD:(g + 1) * HD].reshape(NDC, 128, HD)
                .transpose(1, 0, 2).reshape(128, D).astype(BF)),
            "wo": np.ascontiguousarray(Wo[g * GW:(g + 1) * GW, :].astype(BF)),
            "cosq": COSQ, "sinq": SINQ, "cosk": COSK, "sink": SINK,
            "maskp": maskp,
            "ones128h": np.ones((128, 128), dtype=BF),
            "epsp": np.full((128, 1), EPS, dtype=np.float32),
        })

    global _LAST_IN_MAPS
    _LAST_IN_MAPS = in_maps
    res = run_bass_kernel_spmd(nc, in_maps, list(range(NCORES)))

    out = np.zeros((B, S, D), dtype=np.float32)
    for b in range(B):
        oT = np.concatenate(
            [np.asarray(res.results[4 * b + g]["oshard"]).astype(np.float32)
             for g in range(4)], axis=0)
        out[b, pos_s[b], :] = oT.T + bo[None, :]
    return out
